# revision 1
# baseline (speedup 1.0000x reference)
"""Correlation (cost volume) kernel for Trainium2, 8-core data parallel.

Math (matches the reference):
  x1 = proj(input1), x2 = proj(input2)  (1x1 conv = per-pixel channel matmul)
  x2p = zero-pad(x2, 4 on each spatial side)
  out[b, di*9+dj, i, j] = sum_f x1[b,f,i,j] * x2p[b,f,i+di,j+dj] / sqrt(128)

Device strategy (per core, 4 batches each):
  - projections as [128c x 128f] matmuls (scale 128**-0.25 folded into W on
    both sides so the final /sqrt(128) is free)
  - correlation as banded matmuls: per output row i, three concurrent
    32-column-strip matmuls (tile_position col groups), stationary =
    projected x1 pixels [128c, 32j], moving = padded projected x2 window
    [128c, 9di x 40m] (40 = 32 + 8 slack for dj) -> PSUM band [96j, 360]
  - band tiles go back to DRAM; the per-partition diagonal de-skew
    out[..., dj] = band[..., (j%32)+dj] is done on the host (a gather the
    on-chip engines cannot express: per-partition offsets).

All matmul operands are bf16 (PSUM accumulates fp32).
"""
import math

import numpy as np
import ml_dtypes

import concourse.bass as bass
import concourse.bacc as bacc
import concourse.tile as tile
import concourse.mybir as mybir
from concourse.bass_utils import run_bass_kernel_spmd

B, C, H, W = 32, 128, 96, 96
NCORES = 8
BLOC = B // NCORES          # 4 batches per core
PATCH = 9
R = PATCH // 2              # 4
PH, PW = H + 2 * R, W + 2 * R  # 104 x 104 padded
NPIX = H * W                # 9216
PCHUNK = 384                # projection chunk: 4 image rows (384 px)
NCHUNK = NPIX // PCHUNK     # 24 exactly
WIN = 40                    # moving window per j-strip (32 + 8)
BAND = PATCH * WIN          # 360 band columns per output row
IGROUP = 8                  # output rows batched per SBUF band tile / DMA
OUT_DT = mybir.dt.bfloat16  # band DMA dtype (fp32 PSUM rounded once)

_cache: dict = {}


def _build_program():
    nc = bacc.Bacc(target_bir_lowering=False)
    bf = mybir.dt.bfloat16
    f32 = mybir.dt.float32

    x1d = nc.declare_dram_parameter("x1", [BLOC, C, NPIX], bf, isOutput=False)
    x2d = nc.declare_dram_parameter("x2", [BLOC, C, NPIX], bf, isOutput=False)
    wtd = nc.declare_dram_parameter("wt", [C, C], bf, isOutput=False)
    bd = nc.declare_dram_parameter("bias", [C, 1], f32, isOutput=False)
    bandd = nc.declare_dram_parameter(
        "band", [BLOC, H // IGROUP, H, IGROUP * BAND], OUT_DT, isOutput=True
    )

    with tile.TileContext(nc) as tc:
        with (
            tc.tile_pool(name="consts", bufs=1) as consts,
            tc.tile_pool(name="imgs", bufs=2) as imgs,
            tc.tile_pool(name="feats", bufs=2) as feats,
            tc.tile_pool(name="bands", bufs=4) as bands,
            tc.tile_pool(name="pps", bufs=3, space="PSUM") as pps,
            tc.tile_pool(name="bps", bufs=5, space="PSUM") as bps,
        ):
            wt = consts.tile([C, C], bf, tag="wt")
            nc.sync.dma_start(out=wt[:, :], in_=wtd[:, :])
            bias = consts.tile([C, 1], f32, tag="bias")
            nc.sync.dma_start(out=bias[:, :], in_=bd[:, :])

            ncopy = 0

            def copy(dst, src, add_bias):
                # split PSUM->SBUF copy load between DVE (4/9) and ACT (5/9)
                nonlocal ncopy
                ncopy += 1
                if ncopy % 9 < 4:
                    if add_bias:
                        nc.vector.tensor_scalar_add(dst, src, bias[:, :])
                    else:
                        nc.vector.tensor_copy(dst, src)
                else:
                    if add_bias:
                        nc.scalar.activation(
                            dst, src, mybir.ActivationFunctionType.Identity,
                            bias=bias[:, :],
                        )
                    else:
                        nc.scalar.copy(dst, src)

            for b in range(BLOC):
                x1t = imgs.tile([C, NPIX], bf, tag="x1")
                nc.sync.dma_start(out=x1t[:, :], in_=x1d[b, :, :])
                x2t = imgs.tile([C, NPIX], bf, tag="x2")
                nc.sync.dma_start(out=x2t[:, :], in_=x2d[b, :, :])

                y1 = feats.tile([C, NPIX], bf, tag="y1")
                z2 = feats.tile([C, PH * PW], bf, tag="z2")
                z2v = z2[:, :].rearrange("c (r w) -> c r w", w=PW)
                # zero the pad frame (gpsimd; interior is fully overwritten)
                nc.gpsimd.memset(z2v[:, 0:R, :], 0.0)
                nc.gpsimd.memset(z2v[:, R + H:PH, :], 0.0)
                nc.gpsimd.memset(z2v[:, R:R + H, 0:R], 0.0)
                nc.gpsimd.memset(z2v[:, R:R + H, R + W:PW], 0.0)

                for k in range(NCHUNK):
                    sl = bass.ts(k, PCHUNK)
                    p1 = pps.tile([C, PCHUNK], f32, tag="pp")
                    nc.tensor.matmul(p1[:, :], wt[:, :], x1t[:, sl],
                                     start=True, stop=True)
                    copy(y1[:, sl], p1[:, :], True)
                    p2 = pps.tile([C, PCHUNK], f32, tag="pp")
                    nc.tensor.matmul(p2[:, :], wt[:, :], x2t[:, sl],
                                     start=True, stop=True)
                    p2v = p2[:, :].rearrange("c (r w) -> c r w", w=W)
                    copy(z2v[:, R + 4 * k:R + 4 * k + 4, R:R + W], p2v, True)

                y1v = y1[:, :].rearrange("c (i j) -> c i j", j=W)
                for g in range(H // IGROUP):
                    bt = bands.tile([H, IGROUP * BAND], OUT_DT, tag="bt")
                    for s in range(IGROUP):
                        i = IGROUP * g + s
                        pb = bps.tile([C, BAND], f32, tag="pb")
                        for jb in range(3):
                            nc.tensor.matmul(
                                pb[32 * jb:32 * jb + 32, :],
                                y1v[:, i, 32 * jb:32 * jb + 32],
                                z2v[:, i:i + PATCH, 32 * jb:32 * jb + WIN],
                                start=True, stop=True,
                                tile_position=(0, 32 * jb),
                            )
                        copy(bt[:, bass.ts(s, BAND)], pb[0:H, :], False)
                    nc.sync.dma_start(out=bandd[b, g, :, :], in_=bt[:, :])

    nc.compile()
    return nc


def kernel(input1, input2, proj_w, proj_b):
    if "nc" not in _cache:
        _cache["nc"] = _build_program()
    nc = _cache["nc"]

    s = float(C) ** -0.25  # applied to both projections -> 1/sqrt(C) total
    wt = np.ascontiguousarray((proj_w.astype(np.float64) * s).T).astype(
        ml_dtypes.bfloat16
    )
    bias = (proj_b.astype(np.float64) * s).astype(np.float32).reshape(C, 1)

    in_maps = []
    for k in range(NCORES):
        sl = slice(BLOC * k, BLOC * (k + 1))
        in_maps.append({
            "x1": np.ascontiguousarray(input1[sl]).reshape(BLOC, C, NPIX)
                    .astype(ml_dtypes.bfloat16),
            "x2": np.ascontiguousarray(input2[sl]).reshape(BLOC, C, NPIX)
                    .astype(ml_dtypes.bfloat16),
            "wt": wt,
            "bias": bias,
        })

    res = run_bass_kernel_spmd(nc, in_maps, list(range(NCORES)))

    # host de-skew: out[b, di*9+dj, i, j] = band[b, i, j, di, (j%32)+dj]
    j = np.arange(W)
    idx = (j % 32)[:, None] + np.arange(PATCH)[None, :]        # [96, 9]
    idx6 = np.broadcast_to(idx[None, None, :, None, :],
                           (BLOC, H, W, PATCH, PATCH))
    outs = []
    for k in range(NCORES):
        band = np.asarray(res.results[k]["band"], dtype=np.float32)
        # [BLOC, 24, j, s, di, m] -> [BLOC, i, j, di, m]
        band = band.reshape(BLOC, H // IGROUP, H, IGROUP, PATCH, WIN)
        band = band.transpose(0, 1, 3, 2, 4, 5).reshape(BLOC, H, W, PATCH, WIN)
        sel = np.take_along_axis(band, idx6, axis=-1)          # [b,i,j,di,dj]
        outs.append(sel.transpose(0, 3, 4, 1, 2).reshape(BLOC, PATCH * PATCH, H, W))
    return np.concatenate(outs, axis=0)



# revision 24
# speedup vs baseline: 1.9076x; 1.9076x over previous
"""Correlation (cost volume) kernel for Trainium2, 8-core data parallel.

Math (matches the reference):
  x1 = proj(input1), x2 = proj(input2)  (1x1 conv = per-pixel channel matmul)
  out[b, di*9+dj, i, j] = sum_f x1[b,f,i,j] * x2p[b,f,i+di,j+dj] / sqrt(128)
Since proj is linear, corr = x1^T (W^T W / sqrt(128)) x2 (+ bias terms that
vanish for the zero bias used by setup_inputs; handled on the host if ever
nonzero).  So the device projects ONLY x2 (by M = W^T W / sqrt(128)) and
correlates raw x1 against the projected, zero-padded z2.

Device strategy (per core, 4 batches each):
  - z2 = M @ x2 written into a zero-padded [128c, 104, 104] SBUF image.
  - correlation as block matmuls: stationary = raw x1 pixel block
    [128c, G=16 rows x S=8 cols] (128 stationary columns), moving = z2
    window [128c, (G+8)=24 rows x (S+8)=16 cols] (384 cols).  Each PSUM
    tile [128, 384] holds, for pixel partition p=(gi,sj), the products
    x1[p] . z2[gi+di, sj+dj] at column (gi+di)*16 + (sj+dj).
  - PSUM tiles are drained (bf16) into a per-batch band buffer with
    SLAB-MAJOR layout: band col = r*(T*16) + t*16 + w  (r = z2 row offset,
    t = tile, w = window col).  Per row-in-group gi only slabs gi..gi+8
    are useful, and in this layout they are CONTIGUOUS per partition, so
    one DMA per gi ships exactly those 9 slabs at full DMA efficiency.
  - host de-skew: out[.., dj] = band[.., sj+dj] (a per-partition-offset
    gather the on-chip engines cannot express).

All matmul operands are bf16 (PSUM accumulates fp32).
"""
import numpy as np
import ml_dtypes

import concourse.bass as bass
import concourse.bacc as bacc
import concourse.tile as tile
import concourse.mybir as mybir
from concourse.bass_utils import run_bass_kernel_spmd

B, C, H, W = 32, 128, 96, 96
NCORES = 8
BLOC = B // NCORES          # 4 batches per core
PATCH = 9
R = PATCH // 2              # 4
PH, PW = H + 2 * R, W + 2 * R  # 104 x 104 padded
NPIX = H * W                # 9216
PCHUNK = 384                # projection chunk: 4 image rows (384 px)
NCHUNK = NPIX // PCHUNK     # 24 exactly

G = 16                      # stationary block rows
S = 8                       # stationary block cols (G*S = 128 partitions)
NGG = H // G                # 6 row groups
NST = W // S                # 12 col strips
T = NGG * NST               # 72 tiles per batch
WW = S + 8                  # 16 moving window cols per tile
NSLAB = G + 8               # 24 moving window rows (slabs) per tile
TC = NSLAB * WW             # 384 psum cols per tile
NH = 2                      # band halves (double-buffered drain)
TH = T // NH                # 36 tiles per half
TW = TH * WW                # 576 band cols per slab per half
BANDC = NSLAB * TW          # 13824 band cols per partition per half
NPAIR = G // 2              # 8 gi-pair out-DMAs per half
PSLAB = PATCH + 1           # 10 slabs shipped per gi pair
OUT_DT = mybir.dt.bfloat16

_cache: dict = {}


def _build_program():
    nc = bacc.Bacc(target_bir_lowering=False)
    bf = mybir.dt.bfloat16
    f32 = mybir.dt.float32

    x1d = nc.declare_dram_parameter("x1", [C, BLOC * NPIX], bf,
                                    isOutput=False)
    x2d = nc.declare_dram_parameter("x2", [BLOC, C, NPIX], bf, isOutput=False)
    wtd = nc.declare_dram_parameter("wt", [C, C], bf, isOutput=False)
    bandd = nc.declare_dram_parameter(
        "band", [BLOC, NH, NPAIR, 2 * S, PSLAB, TH, WW], OUT_DT, isOutput=True
    )

    with tile.TileContext(nc) as tc:
        with (
            tc.tile_pool(name="consts", bufs=1) as consts,
            tc.tile_pool(name="imgs", bufs=2) as imgs,
            tc.tile_pool(name="z2s", bufs=2) as z2s,
            tc.tile_pool(name="bands", bufs=1) as bands,
            tc.tile_pool(name="ps", bufs=4, space="PSUM") as ps,
        ):
            wt = consts.tile([C, C], bf, tag="wt")
            nc.sync.dma_start(out=wt[:, :], in_=wtd[:, :])

            # x1 stays fully resident; load upfront so input DMAs never
            # wait on compute.  Order: x2(0), x1q0, x2(1), x1q1.. so proj
            # of batches 0/1 can start as early as possible.
            x1a = consts.tile([C, BLOC * NPIX], bf, tag="x1a")
            x2ts = []
            for b in range(2):
                x2t = imgs.tile([C, NPIX], bf, tag="x2")
                nc.sync.dma_start(out=x2t[:, :], in_=x2d[b, :, :])
                x2ts.append(x2t)
                nc.sync.dma_start(
                    out=x1a[:, bass.ts(b, NPIX)],
                    in_=x1d[:, bass.ts(b, NPIX)],
                )
            for b in range(2, BLOC):
                nc.sync.dma_start(
                    out=x1a[:, bass.ts(b, NPIX)],
                    in_=x1d[:, bass.ts(b, NPIX)],
                )

            # least-loaded rotation of PSUM->SBUF drains over DVE/ACT
            # (gpsimd cannot access PSUM), using each engine's cost model
            eng_load = [0.0, 0.0]

            def copy(dst, src):
                n = src.free_size()
                costs = (1.042 * n + 125, 0.833 * n + 143)
                i = min(range(2), key=lambda e: eng_load[e] + costs[e])
                eng_load[i] += costs[i]
                if i == 0:
                    nc.vector.tensor_copy(dst, src)
                else:
                    nc.scalar.copy(dst, src)

            for b in range(BLOC):
                x2t = x2ts[b]

                z2 = z2s.tile([C, PH * PW], bf, tag="z2")
                z2v = z2[:, :].rearrange("c (r w) -> c r w", w=PW)
                if b < 2:
                    # zero the pad frame once per buffer (interior is
                    # always fully overwritten; pads stay zero after)
                    nc.gpsimd.memset(z2v[:, 0:R, :], 0.0)
                    nc.gpsimd.memset(z2v[:, R + H:PH, :], 0.0)
                    nc.gpsimd.memset(z2v[:, R:R + H, 0:R], 0.0)
                    nc.gpsimd.memset(z2v[:, R:R + H, R + W:PW], 0.0)

                # z2 interior = M @ x2, chunked 4 image rows per matmul,
                # two chunks per PSUM tile / drain copy
                for k2 in range(NCHUNK // 2):
                    p2 = ps.tile([C, 1024], f32, tag="ps")
                    p2h = p2[:, :].rearrange("c (h n) -> c h n", h=2)
                    for h in range(2):
                        k = 2 * k2 + h
                        nc.tensor.matmul(p2h[:, h, 0:PCHUNK], wt[:, :],
                                         x2t[:, bass.ts(k, PCHUNK)],
                                         start=True, stop=True)
                    p2v = p2h[:, :, 0:PCHUNK].rearrange(
                        "c h (r w) -> c h r w", w=W)
                    dst = z2v[:, R + 8 * k2:R + 8 * k2 + 8, R:R + W]
                    copy(dst.rearrange("c (h r) w -> c h r w", h=2), p2v)

                # prefetch next x2 (double-buffered)
                if b + 2 < BLOC:
                    x2n = imgs.tile([C, NPIX], bf, tag="x2")
                    nc.sync.dma_start(out=x2n[:, :], in_=x2d[b + 2, :, :])
                    x2ts.append(x2n)

                # x1 arrives host-pre-blocked: 128 contiguous stationary
                # cols (gi*S+sj) per tile t, so ldweights sees one free dim
                x1b = x1a[:, bass.ts(b, NPIX)]

                for half in range(NH):
                    bt = bands.tile([C, BANDC], OUT_DT, tag=f"bt{half}")
                    # band viewed [p, t, r, w] (t stride WW, r stride TW)
                    btv = bt[:, :].rearrange("p (r t w) -> p t r w",
                                             t=TH, w=WW)
                    for tp in range(TH // 2):
                        pb = ps.tile([C, 1024], f32, tag="ps")
                        pbh = pb[:, :].rearrange("p (h n) -> p h n", h=2)
                        for h in range(2):
                            t = half * TH + 2 * tp + h
                            gg, st = divmod(t, NST)
                            nc.tensor.matmul(
                                pbh[:, h, 0:TC],
                                x1b[:, bass.ts(t, G * S)],
                                z2v[:, G * gg:G * gg + NSLAB,
                                    S * st:S * st + WW],
                                start=True, stop=True,
                            )
                        src = pbh[:, :, 0:TC].rearrange(
                            "p h (r w) -> p h r w", w=WW)
                        copy(btv[:, 2 * tp:2 * tp + 2, :, :], src)

                    for p in range(NPAIR):
                        nc.sync.dma_start(
                            out=bandd[b, half, p, :, :, :, :],
                            in_=bt[2 * S * p:2 * S * (p + 1),
                                   TW * 2 * p:TW * (2 * p + PSLAB)],
                        )

    nc.compile()
    return nc


def kernel(input1, input2, proj_w, proj_b):
    if "nc" not in _cache:
        _cache["nc"] = _build_program()
    nc = _cache["nc"]

    w64 = proj_w.astype(np.float64)
    m = (w64.T @ w64) / np.sqrt(C)            # symmetric; scale folded in
    wt = np.ascontiguousarray(m).astype(ml_dtypes.bfloat16)

    in_maps = []
    for k in range(NCORES):
        sl = slice(BLOC * k, BLOC * (k + 1))
        x1blk = (input1[sl].reshape(BLOC, C, NGG, G, NST, S)
                 .transpose(1, 0, 2, 4, 3, 5).reshape(C, BLOC * NPIX))
        in_maps.append({
            "x1": x1blk.astype(ml_dtypes.bfloat16),
            "x2": np.ascontiguousarray(input2[sl]).reshape(BLOC, C, NPIX)
                    .astype(ml_dtypes.bfloat16),
            "wt": wt,
        })

    res = run_bass_kernel_spmd(nc, in_maps, list(range(NCORES)))

    # host de-skew: band[b, h, p, (gh,sj), rr, (gg_l,st), w] holds the
    # correlation of pixel (i, j) = ((h*3+gg_l)*G + 2p+gh, st*S+sj) with
    # displacement (di, dj) at rr = di+gh, w = sj+dj.
    gh = np.arange(2)[:, None, None, None]
    sj = np.arange(S)[None, :, None, None]
    di = np.arange(PATCH)[None, None, :, None]
    dj = np.arange(PATCH)[None, None, None, :]
    outs = []
    for k in range(NCORES):
        band = np.asarray(res.results[k]["band"], dtype=np.float32)
        arr = band.reshape(BLOC, NH, NPAIR, 2, S, PSLAB, NGG // NH, NST, WW)
        sel = arr[:, :, :, gh, sj, di + gh, :, :, sj + dj]
        # sel: [gh, sj, di, dj, b, h, p, gg_l, st]
        sel = sel.transpose(4, 2, 3, 5, 7, 6, 0, 8, 1).reshape(
            BLOC, PATCH * PATCH, H, W)
        outs.append(sel)
    out = np.concatenate(outs, axis=0)

    if np.any(proj_b != 0):
        # corr += (u.x1[p] + u.x2[q] + b.b)/sqrt(C), u = W^T b, q = shifted
        u = (w64.T @ proj_b.astype(np.float64)).astype(np.float32)
        s1 = np.einsum("c,bcp->bp", u, input1.reshape(B, C, NPIX))
        s1 = s1.reshape(B, 1, H, W)
        s2 = np.einsum("c,bcp->bp", u, input2.reshape(B, C, NPIX))
        s2 = s2.reshape(B, H, W)
        s2p = np.zeros((B, PH, PW), dtype=np.float32)
        s2p[:, R:R + H, R:R + W] = s2
        shifts = np.stack([
            s2p[:, di:di + H, dj:dj + W]
            for di in range(PATCH) for dj in range(PATCH)
        ], axis=1)
        bb = float(proj_b.astype(np.float64) @ proj_b.astype(np.float64))
        out = out + (s1 + shifts + bb) / np.float32(np.sqrt(C))

    return out


# revision 33
# speedup vs baseline: 2.1705x; 1.1378x over previous
"""Correlation (cost volume) kernel for Trainium2, 8-core data parallel.

Math (matches the reference):
  x1 = proj(input1), x2 = proj(input2)  (1x1 conv = per-pixel channel matmul)
  out[b, di*9+dj, i, j] = sum_f x1[b,f,i,j] * x2p[b,f,i+di,j+dj] / sqrt(128)
Since proj is linear, corr = x1^T (W^T W / sqrt(128)) x2 (+ bias terms that
vanish for the zero bias used by setup_inputs; handled on the host if ever
nonzero).  So the device projects ONLY x2 (by M = W^T W / sqrt(128)) and
correlates raw x1 against the projected, zero-padded z2.

Device strategy (per core, 4 batches each):
  - z2 = M @ x2 written into a zero-padded [128c, 104, 104] SBUF image.
  - correlation as block matmuls: stationary = raw x1 pixel block
    [128c, G=16 rows x S=8 cols] (128 stationary columns), moving = z2
    window [128c, (G+8)=24 rows x (S+8)=16 cols] (384 cols).  Each PSUM
    tile [128, 384] holds, for pixel partition p=(gi,sj), the products
    x1[p] . z2[gi+di, sj+dj] at column (gi+di)*16 + (sj+dj).
  - PSUM tiles are drained (bf16) into a per-batch band buffer with
    SLAB-MAJOR layout: band col = r*(T*16) + t*16 + w  (r = z2 row offset,
    t = tile, w = window col).  Per row-in-group gi only slabs gi..gi+8
    are useful, and in this layout they are CONTIGUOUS per partition, so
    one DMA per gi ships exactly those 9 slabs at full DMA efficiency.
  - host de-skew: out[.., dj] = band[.., sj+dj] (a per-partition-offset
    gather the on-chip engines cannot express).

All matmul operands are bf16 (PSUM accumulates fp32).
"""
import numpy as np
import ml_dtypes

import concourse.bass as bass
import concourse.bacc as bacc
import concourse.tile as tile
import concourse.mybir as mybir
from concourse.bass_utils import run_bass_kernel_spmd

B, C, H, W = 32, 128, 96, 96
NCORES = 8
BLOC = B // NCORES          # 4 batches per core
PATCH = 9
R = PATCH // 2              # 4
PH, PW = H + 2 * R, W + 2 * R  # 104 x 104 padded
NPIX = H * W                # 9216
PCHUNK = 384                # projection chunk: 4 image rows (384 px)
NCHUNK = NPIX // PCHUNK     # 24 exactly

G = 16                      # stationary block rows
S = 8                       # stationary block cols (G*S = 128 partitions)
ZW = W + 2 * R              # z2 row width 104 (horizontal pad only)
NGG = H // G                # 6 row groups
NST = W // S                # 12 col strips
T = NGG * NST               # 72 tiles per batch
WW = S + 8                  # 16 moving window cols per tile
NSLAB = G + 8               # 24 moving window rows (slabs) per tile
TC = NSLAB * WW             # 384 psum cols per tile
NH = 2                      # band halves (double-buffered drain)
TH = T // NH                # 36 tiles per half
TW = TH * WW                # 576 band cols per slab per half
BANDC = NSLAB * TW          # 13824 band cols per partition per half
NPAIR = G // 2              # 8 gi-pair out-DMAs per half
PSLAB = PATCH + 1           # 10 slabs shipped per gi pair
OUT_DT = mybir.dt.bfloat16

_cache: dict = {}


def _build_program():
    nc = bacc.Bacc(target_bir_lowering=False)
    bf = mybir.dt.bfloat16
    f32 = mybir.dt.float32

    x1d = nc.declare_dram_parameter("x1", [C, BLOC * NPIX], bf,
                                    isOutput=False)
    x2d = nc.declare_dram_parameter("x2", [BLOC, C, NPIX], bf, isOutput=False)
    wtd = nc.declare_dram_parameter("wt", [C, C], bf, isOutput=False)
    bandd = nc.declare_dram_parameter(
        "band", [BLOC, NH, NPAIR, 2 * S, PSLAB, TH, WW], OUT_DT, isOutput=True
    )

    with tile.TileContext(nc) as tc:
        with (
            tc.tile_pool(name="consts", bufs=1) as consts,
            tc.tile_pool(name="imgs", bufs=2) as imgs,
            tc.tile_pool(name="z2s", bufs=2) as z2s,
            tc.tile_pool(name="bands", bufs=1) as bands,
            tc.tile_pool(name="ps", bufs=4, space="PSUM") as ps,
        ):
            wt = consts.tile([C, C], bf, tag="wt")
            nc.sync.dma_start(out=wt[:, :], in_=wtd[:, :])

            # x1 stays fully resident.  The DMA unit serves transfers in
            # request order, so emit inputs just-in-time in small chunks
            # (eighths) to avoid head-of-line blocking the band out-DMAs.
            HPIX = NPIX // 2
            x1a = consts.tile([C, BLOC * NPIX], bf, tag="x1a")

            def x1_eighth(e):
                nc.sync.dma_start(out=x1a[:, bass.ts(e, HPIX)],
                                  in_=x1d[:, bass.ts(e, HPIX)])

            def dma_x2(b):
                x2t = imgs.tile([C, NPIX], bf, tag="x2")
                nc.sync.dma_start(out=x2t[:, 0:HPIX], in_=x2d[b, :, 0:HPIX])
                nc.sync.dma_start(out=x2t[:, HPIX:NPIX],
                                  in_=x2d[b, :, HPIX:NPIX])
                return x2t

            x2ts = []
            for b in range(2):
                x2ts.append(dma_x2(b))
                x1_eighth(2 * b)
                x1_eighth(2 * b + 1)

            # least-loaded rotation of PSUM->SBUF drains over DVE/ACT
            # (gpsimd cannot access PSUM), using each engine's cost model
            eng_load = [0.0, 0.0]

            def copy(dst, src):
                n = src.free_size()
                costs = (1.042 * n + 125, 0.833 * n + 143)
                i = min(range(2), key=lambda e: eng_load[e] + costs[e])
                eng_load[i] += costs[i]
                if i == 0:
                    nc.vector.tensor_copy(dst, src)
                else:
                    nc.scalar.copy(dst, src)

            for b in range(BLOC):
                x2t = x2ts[b]

                # z2 is horizontally padded only; vertical out-of-image
                # rows are skipped by clamped matmul windows and the
                # corresponding outputs zeroed on the host
                z2 = z2s.tile([C, H * ZW], bf, tag="z2")
                z2v = z2[:, :].rearrange("c (r w) -> c r w", w=ZW)
                if b < 2:
                    # zero the left/right pad strips once per buffer
                    # (interior is always fully overwritten)
                    nc.gpsimd.memset(z2v[:, :, 0:R], 0.0)
                    nc.gpsimd.memset(z2v[:, :, R + W:ZW], 0.0)

                # z2 interior = M @ x2, chunked 4 image rows per matmul,
                # two chunks per PSUM tile / drain copy
                for k2 in range(NCHUNK // 2):
                    p2 = ps.tile([C, 1024], f32, tag="ps")
                    p2h = p2[:, :].rearrange("c (h n) -> c h n", h=2)
                    for h in range(2):
                        k = 2 * k2 + h
                        nc.tensor.matmul(p2h[:, h, 0:PCHUNK], wt[:, :],
                                         x2t[:, bass.ts(k, PCHUNK)],
                                         start=True, stop=True)
                    p2v = p2h[:, :, 0:PCHUNK].rearrange(
                        "c h (r w) -> c h r w", w=W)
                    dst = z2v[:, 8 * k2:8 * k2 + 8, R:R + W]
                    copy(dst.rearrange("c (h r) w -> c h r w", h=2), p2v)

                # prefetch next x2 (double-buffered)
                if b + 2 < BLOC:
                    x2ts.append(dma_x2(b + 2))

                # x1 arrives host-pre-blocked: 128 contiguous stationary
                # cols (gi*S+sj) per tile t, so ldweights sees one free dim
                x1b = x1a[:, bass.ts(b, NPIX)]

                for half in range(NH):
                    bt = bands.tile([C, BANDC], OUT_DT, tag=f"bt{half}")
                    # band viewed [p, t, r, w] (t stride WW, r stride TW)
                    btv = bt[:, :].rearrange("p (r t w) -> p t r w",
                                             t=TH, w=WW)
                    for tp in range(TH // 2):
                        gg = (half * TH + 2 * tp) // NST
                        # clamp the moving window to in-image rows; band
                        # slabs outside [rb0, rb0+nsl) stay garbage and the
                        # host zeroes the matching out-of-image outputs
                        r0 = max(0, G * gg - R)
                        nsl = min(H, G * gg + NSLAB - R) - r0
                        rb0 = r0 - (G * gg - R)
                        pb = ps.tile([C, 1024], f32, tag="ps")
                        pbh = pb[:, :].rearrange("p (h n) -> p h n", h=2)
                        for h in range(2):
                            t = half * TH + 2 * tp + h
                            st = t % NST
                            nc.tensor.matmul(
                                pbh[:, h, 0:nsl * WW],
                                x1b[:, bass.ts(t, G * S)],
                                z2v[:, r0:r0 + nsl,
                                    S * st:S * st + WW],
                                start=True, stop=True,
                            )
                        src = pbh[:, :, 0:nsl * WW].rearrange(
                            "p h (r w) -> p h r w", w=WW)
                        copy(btv[:, 2 * tp:2 * tp + 2,
                                 rb0:rb0 + nsl, :], src)

                    for p in range(NPAIR):
                        nc.sync.dma_start(
                            out=bandd[b, half, p, :, :, :, :],
                            in_=bt[2 * S * p:2 * S * (p + 1),
                                   TW * 2 * p:TW * (2 * p + PSLAB)],
                        )
                    # stream the remaining x1 eighths between out-DMA groups
                    e = 4 + 2 * b + half
                    if b < 2:
                        x1_eighth(e)

    nc.compile()
    return nc


def kernel(input1, input2, proj_w, proj_b):
    if "nc" not in _cache:
        _cache["nc"] = _build_program()
    nc = _cache["nc"]

    w64 = proj_w.astype(np.float64)
    m = (w64.T @ w64) / np.sqrt(C)            # symmetric; scale folded in
    wt = np.ascontiguousarray(m).astype(ml_dtypes.bfloat16)

    in_maps = []
    for k in range(NCORES):
        sl = slice(BLOC * k, BLOC * (k + 1))
        x1blk = (input1[sl].reshape(BLOC, C, NGG, G, NST, S)
                 .transpose(1, 0, 2, 4, 3, 5).reshape(C, BLOC * NPIX))
        in_maps.append({
            "x1": x1blk.astype(ml_dtypes.bfloat16),
            "x2": np.ascontiguousarray(input2[sl]).reshape(BLOC, C, NPIX)
                    .astype(ml_dtypes.bfloat16),
            "wt": wt,
        })

    res = run_bass_kernel_spmd(nc, in_maps, list(range(NCORES)))

    # host de-skew: band[b, h, p, (gh,sj), rr, (gg_l,st), w] holds the
    # correlation of pixel (i, j) = ((h*3+gg_l)*G + 2p+gh, st*S+sj) with
    # displacement (di, dj) at rr = di+gh, w = sj+dj.
    gh = np.arange(2)[:, None, None, None]
    sj = np.arange(S)[None, :, None, None]
    di = np.arange(PATCH)[None, None, :, None]
    dj = np.arange(PATCH)[None, None, None, :]
    outs = []
    for k in range(NCORES):
        band = np.asarray(res.results[k]["band"], dtype=np.float32)
        arr = band.reshape(BLOC, NH, NPAIR, 2, S, PSLAB, NGG // NH, NST, WW)
        sel = arr[:, :, :, gh, sj, di + gh, :, :, sj + dj]
        # sel: [gh, sj, di, dj, b, h, p, gg_l, st]
        sel = sel.transpose(4, 2, 3, 5, 7, 6, 0, 8, 1).reshape(
            BLOC, PATCH * PATCH, H, W)
        outs.append(sel)
    out = np.concatenate(outs, axis=0)

    # zero the vertically out-of-image outputs (reference zero-pads; the
    # device skips those rows so the band holds garbage there)
    for di in range(PATCH):
        dsl = slice(di * PATCH, (di + 1) * PATCH)
        if di < R:
            out[:, dsl, :R - di, :] = 0.0
        if di > R:
            out[:, dsl, H + R - di:, :] = 0.0

    if np.any(proj_b != 0):
        # corr += (u.x1[p] + u.x2[q] + b.b)/sqrt(C), u = W^T b, q = shifted
        u = (w64.T @ proj_b.astype(np.float64)).astype(np.float32)
        s1 = np.einsum("c,bcp->bp", u, input1.reshape(B, C, NPIX))
        s1 = s1.reshape(B, 1, H, W)
        s2 = np.einsum("c,bcp->bp", u, input2.reshape(B, C, NPIX))
        s2 = s2.reshape(B, H, W)
        s2p = np.zeros((B, PH, PW), dtype=np.float32)
        s2p[:, R:R + H, R:R + W] = s2
        shifts = np.stack([
            s2p[:, di:di + H, dj:dj + W]
            for di in range(PATCH) for dj in range(PATCH)
        ], axis=1)
        bb = float(proj_b.astype(np.float64) @ proj_b.astype(np.float64))
        out = out + (s1 + shifts + bb) / np.float32(np.sqrt(C))

    return out
